# revision 8
# baseline (speedup 1.0000x reference)
"""KNN classifier kernel for Trainium2 (8 NeuronCores, Bass/Tile).

Problem (nn_KNNClassifier): given queries x [4096, 512], train bank
x_train [65536, 512], labels y_train [65536] (100 classes), compute for
each query the top-200 neighbors by dot-product similarity, weight them
by exp(sim/0.1), accumulate per-class scores, and return the descending
argsort of class scores -> int32 [4096, 100].

Device strategy (sharding_hint: shard train bank over N across 8 cores):
  - Host reorders x_train columns by class, zero-padding each class to a
    multiple of 256, so every 256-wide column chunk holds one class.
    Each core takes 1/8 of the chunks plus the full query set.
  - Per core: sim = x @ shard^T via float32r matmuls (full PE rate),
    then one DVE max8 per 256-chunk -> top-8 values per (query, chunk).
    Chunk class is known host-side, so no index extraction is needed;
    zero-pad columns yield exact 0.0 values that the host discards.
  - Host gathers 8 * chunks * 8 candidate values per query -- a superset
    of the global top-200 unless a chunk had >8 entries above threshold,
    which is detected (chunk 8th-max >= threshold - slack) and repaired
    by exact recomputation of that chunk (or per-query fallback).
  - float32r is TF32-like (measured |err| <= ~0.022 at K=512); every
    candidate chunk near the top-200 threshold is recomputed exactly on
    host, so the selected top-200 set matches fp32 reference semantics.
  - Final per-class accumulation mimics the reference exactly (fp32 exp
    -> scatter-add -> stable argsort of negated scores).
"""

import os
import sys

for _p in ("/opt/trn_rl_repo",):
    if _p not in sys.path and os.path.isdir(_p):
        sys.path.insert(0, _p)

import numpy as np

import concourse.mybir as mybir
import concourse.tile as tile
from concourse import bacc
from concourse.bass_utils import run_bass_kernel_spmd

# Problem shapes (hardcoded per spec)
B, N, D = 4096, 65536, 512
NUM_CLASSES = 100
KNN_K = 200
KNN_T = 0.1
NCORES = 8

KT = D // 128  # 4 contraction tiles
QB = B // 128  # 32 query blocks of 128
CHUNK = 232  # class-pure chunk width (pads classes ~8%, keeps N>452)
NTILE = 2 * CHUNK  # matmul moving free dim (2 chunks)

SLACK = 0.05  # exact-recompute band around the top-200 threshold
NEG = -1.0e30

_CACHE = {}
LAST_INFO = {}


def _build_program(C, groups):
    """Per-core Bass program: C chunks of 256 columns, streamed in groups
    of `groups[i]` n-tiles (n-tile = 512 cols = 2 chunks)."""
    nc = bacc.Bacc(
        "TRN2", target_bir_lowering=False, debug=False, num_devices=NCORES
    )
    f32 = mybir.dt.float32
    f32r = mybir.dt.float32r

    ncols = C * CHUNK
    cands = C * 8

    xT_d = nc.dram_tensor("xT", (D, B), f32r, kind="ExternalInput").ap()
    wT_d = nc.dram_tensor("wT", (D, ncols), f32r, kind="ExternalInput").ap()
    vals_d = nc.dram_tensor("vals", (B, cands), f32, kind="ExternalOutput").ap()

    from contextlib import ExitStack

    with tile.TileContext(nc) as tc:
        with ExitStack() as ctx:
            xpool = ctx.enter_context(tc.tile_pool(name="xp", bufs=1))
            wpool = ctx.enter_context(tc.tile_pool(name="wp", bufs=2))
            spool = ctx.enter_context(tc.tile_pool(name="sp", bufs=3))
            ppool = ctx.enter_context(tc.tile_pool(name="pp", bufs=2, space="PSUM"))
            opool = ctx.enter_context(tc.tile_pool(name="op", bufs=3))

            xsb = xpool.tile([128, KT * B], f32r, tag="x")

            col0 = 0  # start column of current group
            for gi, gnt in enumerate(groups):
                gcols = gnt * NTILE
                gchunks = gcols // CHUNK
                wt = wpool.tile([128, KT * gcols], f32r, tag="w")
                for k in range(KT):
                    nc.sync.dma_start(
                        wt[:, k * gcols : (k + 1) * gcols],
                        wT_d[k * 128 : (k + 1) * 128, col0 : col0 + gcols],
                    )
                if gi == 0:
                    # xT loads in per-query-block slices so the first
                    # blocks' matmuls start as soon as possible.
                    for b in range(QB):
                        for k in range(KT):
                            nc.sync.dma_start(
                                xsb[:, k * B + b * 128 : k * B + (b + 1) * 128],
                                xT_d[k * 128 : (k + 1) * 128, b * 128 : (b + 1) * 128],
                            )
                for b in range(QB):
                    # PSUM slots are bank-aligned (512 floats); each matmul
                    # writes a 464-wide span inside its own bank.
                    ps = ppool.tile([128, gnt * 512], f32, tag="ps")
                    # k outer: consecutive matmuls share the same stationary
                    # weights (walrus can reuse the loaded weight tile).
                    for k in range(KT):
                        for nt in range(gnt):
                            nc.tensor.matmul(
                                ps[:, nt * 512 : nt * 512 + NTILE],
                                xsb[:, k * B + b * 128 : k * B + (b + 1) * 128],
                                wt[:, k * gcols + nt * NTILE : k * gcols + (nt + 1) * NTILE],
                                start=(k == 0),
                                stop=(k == KT - 1),
                            )
                    sim = spool.tile([128, gnt * 512], f32, tag="sim")
                    nc.scalar.copy(sim[:], ps[:])
                    vt = opool.tile([128, gchunks * 8], f32, tag="v")
                    for ch in range(gchunks):
                        base = (ch // 2) * 512 + (ch % 2) * CHUNK
                        nc.vector.max(
                            vt[:, ch * 8 : (ch + 1) * 8],
                            sim[:, base : base + CHUNK],
                        )
                    nc.sync.dma_start(
                        vals_d[
                            b * 128 : (b + 1) * 128,
                            (col0 // CHUNK) * 8 : (col0 // CHUNK + gchunks) * 8,
                        ],
                        vt[:],
                    )
                col0 += gcols

    nc.compile()
    return nc


def _get_program(C, groups):
    key = (C, tuple(groups))
    if key not in _CACHE:
        _CACHE[key] = _build_program(C, groups)
    return _CACHE[key]


def _plan_layout(y_train):
    """Class-sorted zero-padded column layout.

    Returns (colmap, chunk_class, C, groups):
      colmap: int64 [8*C*CHUNK] -> original x_train row, or -1 for padding
      chunk_class: int64 [8*C] -> class of each global chunk (-1 dummy)
    """
    cnt = np.bincount(y_train, minlength=NUM_CLASSES)
    by_class = np.argsort(y_train, kind="stable")  # rows grouped by class
    cpc = np.maximum((cnt + CHUNK - 1) // CHUNK, 0)  # chunks per class
    total = int(cpc.sum())
    # round chunk count up so chunks/core is even (integral 512 n-tiles)
    T = ((total + 15) // 16) * 16
    C = T // NCORES

    colmap = np.full(T * CHUNK, -1, dtype=np.int64)
    chunk_class = np.full(T, -1, dtype=np.int64)
    pos = 0  # class-group start within by_class
    col = 0
    ch = 0
    for c in range(NUM_CLASSES):
        n = int(cnt[c])
        colmap[col : col + n] = by_class[pos : pos + n]
        nch = int(cpc[c])
        chunk_class[ch : ch + nch] = c
        pos += n
        col += nch * CHUNK
        ch += nch

    nnt = C * CHUNK // NTILE  # n-tiles per core
    groups = [4] * (nnt // 4)
    if nnt % 4:
        groups.append(nnt % 4)
    return colmap, chunk_class, C, groups


def _host_merge(x, x_train, y_train, vals, colmap, chunk_class, C):
    """Exact top-200 -> class scores -> ranking from per-core candidates."""
    x64 = x.astype(np.float64)
    xt64 = x_train.astype(np.float64)
    T = NCORES * C  # global chunk count
    M = T * 8

    V = np.concatenate(list(vals), axis=1).astype(np.float64)  # [B, M]
    V[V == 0.0] = NEG  # zero-pad artifacts (real sims are never exactly 0)

    kth = M - KNN_K
    t0 = np.partition(V, kth, axis=1)[:, kth]  # [B] approx threshold

    # Chunks needing exact recomputation: any candidate within SLACK of
    # the threshold, or chunk 8th-max (possible hidden elements) near it.
    near = V >= (t0[:, None] - SLACK - 0.01)
    lo = V <= (t0[:, None] + SLACK)
    band = near & lo  # candidate needs exact value
    v8 = V.reshape(B, T, 8)[:, :, 7]
    flag = v8 >= (t0[:, None] - SLACK)  # chunk may hide >8 relevant entries
    chunk_band = band.reshape(B, T, 8).any(axis=2) | flag  # [B, T]

    bq, bg = np.nonzero(chunk_band)
    LAST_INFO["recomputed_chunks"] = int(bq.size)
    full_fallback = set()
    if bq.size:
        # Exact sims per (query, chunk) pair, grouped by chunk so each
        # chunk's column matrix is gathered and transposed only once.
        Vr = V.reshape(B, T, 8)
        order = np.argsort(bg, kind="stable")
        bq_s, bg_s = bq[order], bg[order]
        starts = np.searchsorted(bg_s, np.unique(bg_s))
        bounds = list(starts) + [bg_s.size]
        for i in range(len(starts)):
            s, e = bounds[i], bounds[i + 1]
            g = int(bg_s[s])
            qs = bq_s[s:e]
            rows = colmap[g * CHUNK : (g + 1) * CHUNK]
            pad = rows < 0
            Wg = xt64[np.where(pad, 0, rows)].T  # [D, CHUNK]
            exact = x64[qs] @ Wg  # [nq, CHUNK]
            exact[:, pad] = NEG
            thr = t0[qs] - SLACK - 0.005
            nkeep = (exact >= thr[:, None]).sum(axis=1)
            top8 = -np.sort(-exact, axis=1)[:, :8]
            Vr[qs, g] = top8
            for q in qs[nkeep > 8]:
                full_fallback.add(int(q))

    t1 = np.partition(V, kth, axis=1)[:, kth]
    sel = np.argpartition(-V, KNN_K - 1, axis=1)[:, :KNN_K]
    rowix = np.arange(B)[:, None]
    sel_v = V[rowix, sel]

    # Boundary ties -> per-query fallback (argpartition splits arbitrarily)
    vmin = sel_v.min(axis=1)
    tie = (V == vmin[:, None]).sum(axis=1) != (sel_v == vmin[:, None]).sum(axis=1)
    for q in np.nonzero(tie)[0]:
        full_fallback.add(int(q))
    LAST_INFO["fallback_rows"] = len(full_fallback)

    cand_class = np.repeat(chunk_class, 8)  # [M] class per candidate slot
    labels = cand_class[sel]  # [B, K]

    # Pathological guard: if the top-200 threshold ever sits near/below 0,
    # zero-pad dropping could hide real candidates -> recompute those rows.
    for q in np.nonzero(t1 < 1.0)[0]:
        full_fallback.add(int(q))

    scores = np.zeros((B, NUM_CLASSES), dtype=np.float32)
    with np.errstate(over="ignore"):
        w = np.exp(sel_v.astype(np.float32) / np.float32(KNN_T))
    ok = np.ones(B, dtype=bool)
    for q in full_fallback:
        ok[q] = False
    qs = np.nonzero(ok)[0]
    np.add.at(
        scores,
        (np.repeat(qs, KNN_K), labels[qs].ravel()),
        w[qs].ravel(),
    )

    for q in full_fallback:
        sims = xt64 @ x64[q]
        order = np.lexsort((np.arange(N), -sims))[:KNN_K]
        lab = y_train[order]
        with np.errstate(over="ignore"):
            wq = np.exp(sims[order].astype(np.float32) / np.float32(KNN_T))
        np.add.at(scores[q], lab, wq)

    return np.argsort(-scores, axis=1, kind="stable").astype(np.int32)


def kernel(x, x_train, y_train):
    x = np.asarray(x, dtype=np.float32)
    x_train = np.asarray(x_train, dtype=np.float32)
    y_train = np.asarray(y_train).astype(np.int64)

    colmap, chunk_class, C, groups = _plan_layout(y_train)
    nc = _get_program(C, groups)

    ncols_tot = colmap.shape[0]
    xtrP = np.zeros((D, ncols_tot), dtype=np.float32)  # padded, transposed
    real = colmap >= 0
    xtrP[:, real] = x_train.T[:, colmap[real]]

    xT = np.ascontiguousarray(x.T)
    ncols = C * CHUNK
    in_maps = [
        {
            "xT": xT,
            "wT": np.ascontiguousarray(xtrP[:, c * ncols : (c + 1) * ncols]),
        }
        for c in range(NCORES)
    ]

    res = run_bass_kernel_spmd(nc, in_maps, core_ids=list(range(NCORES)))
    LAST_INFO["exec_time_ns"] = res.exec_time_ns
    LAST_INFO["results"] = res

    vals = np.stack([res.results[c]["vals"] for c in range(NCORES)])  # [8, B, C*8]
    return _host_merge(x, x_train, y_train, vals, colmap, chunk_class, C)


# revision 12
# speedup vs baseline: 1.1195x; 1.1195x over previous
"""KNN classifier kernel for Trainium2 (8 NeuronCores, Bass/Tile).

Problem (nn_KNNClassifier): given queries x [4096, 512], train bank
x_train [65536, 512], labels y_train [65536] (100 classes), compute for
each query the top-200 neighbors by dot-product similarity, weight them
by exp(sim/0.1), accumulate per-class scores, and return the descending
argsort of class scores -> int32 [4096, 100].

Device strategy (sharding_hint: shard train bank over N across 8 cores):
  - Host reorders x_train columns by class, zero-padding each class to a
    multiple of 256, so every 256-wide column chunk holds one class.
    Each core takes 1/8 of the chunks plus the full query set.
  - Per core: sim = x @ shard^T via float32r matmuls (full PE rate),
    then one DVE max8 per 256-chunk -> top-8 values per (query, chunk).
    Chunk class is known host-side, so no index extraction is needed;
    zero-pad columns yield exact 0.0 values that the host discards.
  - Host gathers 8 * chunks * 8 candidate values per query -- a superset
    of the global top-200 unless a chunk had >8 entries above threshold,
    which is detected (chunk 8th-max >= threshold - slack) and repaired
    by exact recomputation of that chunk (or per-query fallback).
  - float32r is TF32-like (measured |err| <= ~0.022 at K=512); every
    candidate chunk near the top-200 threshold is recomputed exactly on
    host, so the selected top-200 set matches fp32 reference semantics.
  - Final per-class accumulation mimics the reference exactly (fp32 exp
    -> scatter-add -> stable argsort of negated scores).
"""

import os
import sys

for _p in ("/opt/trn_rl_repo",):
    if _p not in sys.path and os.path.isdir(_p):
        sys.path.insert(0, _p)

import numpy as np

import concourse.mybir as mybir
import concourse.tile as tile
from concourse import bacc
from concourse.bass_utils import run_bass_kernel_spmd

# Problem shapes (hardcoded per spec)
B, N, D = 4096, 65536, 512
NUM_CLASSES = 100
KNN_K = 200
KNN_T = 0.1
NCORES = 8

KT = D // 128  # 4 contraction tiles
QB = B // 128  # 32 query blocks of 128
CHUNK = 228  # class-pure chunk width (pads classes ~8%, keeps N>452)
NTILE = 2 * CHUNK  # matmul moving free dim (2 chunks, one PSUM bank)

SLACK = 0.05  # exact-recompute band around the top-200 threshold
NEG = -1.0e30

_CACHE = {}
LAST_INFO = {}


def _build_program(C, groups):
    """Per-core Bass program: C chunks of CHUNK columns, streamed in groups.

    groups[i] is a list of n-tile widths (each a multiple of CHUNK and at
    most 512, i.e. 1 or 2 chunks). Each n-tile gets its own PSUM bank.
    """
    nc = bacc.Bacc(
        "TRN2", target_bir_lowering=False, debug=False, num_devices=NCORES
    )
    f32 = mybir.dt.float32
    f32r = mybir.dt.float32r

    ncols = C * CHUNK
    cands = C * 8

    xT_d = nc.dram_tensor("xT", (D, B), f32r, kind="ExternalInput").ap()
    wT_d = nc.dram_tensor("wT", (D, ncols), f32r, kind="ExternalInput").ap()
    vals_d = nc.dram_tensor("vals", (B, cands), f32, kind="ExternalOutput").ap()

    from contextlib import ExitStack

    with tile.TileContext(nc) as tc:
        with ExitStack() as ctx:
            xpool = ctx.enter_context(tc.tile_pool(name="xp", bufs=1))
            wpool = ctx.enter_context(tc.tile_pool(name="wp", bufs=2))
            spool = ctx.enter_context(tc.tile_pool(name="sp", bufs=3))
            ppool = ctx.enter_context(tc.tile_pool(name="pp", bufs=2, space="PSUM"))
            opool = ctx.enter_context(tc.tile_pool(name="op", bufs=3))

            xsb = xpool.tile([128, KT * B], f32r, tag="x")

            col0 = 0  # start column of current group
            for gi, gtiles in enumerate(groups):
                gnt = len(gtiles)
                gcols = sum(gtiles)
                gchunks = gcols // CHUNK
                wt = wpool.tile([128, KT * gcols], f32r, tag="w")
                for k in range(KT):
                    if gi == 0:
                        # Interleave xT and group-0 weights per k-slice so
                        # the k=0 matmuls can start after ~4MB of DMA.
                        nc.sync.dma_start(
                            xsb[:, k * B : (k + 1) * B],
                            xT_d[k * 128 : (k + 1) * 128, :],
                        )
                    nc.sync.dma_start(
                        wt[:, k * gcols : (k + 1) * gcols],
                        wT_d[k * 128 : (k + 1) * 128, col0 : col0 + gcols],
                    )
                for b in range(QB):
                    # PSUM slots are bank-aligned (512 floats); each matmul
                    # writes an NTILE-wide span inside its own bank.
                    ps = ppool.tile([128, gnt * 512], f32, tag="ps")
                    for k in range(KT):
                        toff = 0
                        for nt, ntw in enumerate(gtiles):
                            nc.tensor.matmul(
                                ps[:, nt * 512 : nt * 512 + ntw],
                                xsb[:, k * B + b * 128 : k * B + (b + 1) * 128],
                                wt[:, k * gcols + toff : k * gcols + toff + ntw],
                                start=(k == 0),
                                stop=(k == KT - 1),
                            )
                            toff += ntw
                    sim = spool.tile([128, gnt * 512], f32, tag="sim")
                    nc.scalar.copy(sim[:], ps[:])
                    vt = opool.tile([128, gchunks * 8], f32, tag="v")
                    ch = 0
                    for nt, ntw in enumerate(gtiles):
                        for sub in range(ntw // CHUNK):
                            nc.vector.max(
                                vt[:, ch * 8 : (ch + 1) * 8],
                                sim[:, nt * 512 + sub * CHUNK : nt * 512 + (sub + 1) * CHUNK],
                            )
                            ch += 1
                    nc.sync.dma_start(
                        vals_d[
                            b * 128 : (b + 1) * 128,
                            (col0 // CHUNK) * 8 : (col0 // CHUNK + gchunks) * 8,
                        ],
                        vt[:],
                    )
                col0 += gcols

    nc.compile()
    return nc


def _get_program(C, groups):
    key = (C, tuple(tuple(g) for g in groups))
    if key not in _CACHE:
        _CACHE[key] = _build_program(C, groups)
    return _CACHE[key]


def _plan_layout(y_train):
    """Class-sorted zero-padded column layout.

    Returns (colmap, chunk_class, C, groups):
      colmap: int64 [8*C*CHUNK] -> original x_train row, or -1 for padding
      chunk_class: int64 [8*C] -> class of each global chunk (-1 dummy)
    """
    cnt = np.bincount(y_train, minlength=NUM_CLASSES)
    by_class = np.argsort(y_train, kind="stable")  # rows grouped by class
    cpc = np.maximum((cnt + CHUNK - 1) // CHUNK, 0)  # chunks per class
    total = int(cpc.sum())
    T = ((total + NCORES - 1) // NCORES) * NCORES
    C = T // NCORES

    colmap = np.full(T * CHUNK, -1, dtype=np.int64)
    chunk_class = np.full(T, -1, dtype=np.int64)
    pos = 0  # class-group start within by_class
    col = 0
    ch = 0
    for c in range(NUM_CLASSES):
        n = int(cnt[c])
        colmap[col : col + n] = by_class[pos : pos + n]
        nch = int(cpc[c])
        chunk_class[ch : ch + nch] = c
        pos += n
        col += nch * CHUNK
        ch += nch

    # n-tile widths covering C chunks: 2-chunk tiles plus one 1-chunk tail
    tiles = [NTILE] * (C // 2) + ([CHUNK] if C % 2 else [])
    groups = [tiles[i : i + 4] for i in range(0, len(tiles), 4)]
    return colmap, chunk_class, C, groups


def _host_merge(x, x_train, y_train, vals, colmap, chunk_class, C):
    """Exact top-200 -> class scores -> ranking from per-core candidates."""
    x64 = x.astype(np.float64)
    xt64 = x_train.astype(np.float64)
    T = NCORES * C  # global chunk count
    M = T * 8

    V = np.concatenate(list(vals), axis=1).astype(np.float64)  # [B, M]
    V[V == 0.0] = NEG  # zero-pad artifacts (real sims are never exactly 0)

    kth = M - KNN_K
    t0 = np.partition(V, kth, axis=1)[:, kth]  # [B] approx threshold

    # Chunks needing exact recomputation: any candidate within SLACK of
    # the threshold, or chunk 8th-max (possible hidden elements) near it.
    near = V >= (t0[:, None] - SLACK - 0.01)
    lo = V <= (t0[:, None] + SLACK)
    band = near & lo  # candidate needs exact value
    v8 = V.reshape(B, T, 8)[:, :, 7]
    flag = v8 >= (t0[:, None] - SLACK)  # chunk may hide >8 relevant entries
    chunk_band = band.reshape(B, T, 8).any(axis=2) | flag  # [B, T]

    bq, bg = np.nonzero(chunk_band)
    LAST_INFO["recomputed_chunks"] = int(bq.size)
    full_fallback = set()
    if bq.size:
        # Exact sims per (query, chunk) pair, grouped by chunk so each
        # chunk's column matrix is gathered and transposed only once.
        Vr = V.reshape(B, T, 8)
        order = np.argsort(bg, kind="stable")
        bq_s, bg_s = bq[order], bg[order]
        starts = np.searchsorted(bg_s, np.unique(bg_s))
        bounds = list(starts) + [bg_s.size]
        for i in range(len(starts)):
            s, e = bounds[i], bounds[i + 1]
            g = int(bg_s[s])
            qs = bq_s[s:e]
            rows = colmap[g * CHUNK : (g + 1) * CHUNK]
            pad = rows < 0
            Wg = xt64[np.where(pad, 0, rows)].T  # [D, CHUNK]
            exact = x64[qs] @ Wg  # [nq, CHUNK]
            exact[:, pad] = NEG
            thr = t0[qs] - SLACK - 0.005
            nkeep = (exact >= thr[:, None]).sum(axis=1)
            top8 = -np.sort(-exact, axis=1)[:, :8]
            Vr[qs, g] = top8
            for q in qs[nkeep > 8]:
                full_fallback.add(int(q))

    t1 = np.partition(V, kth, axis=1)[:, kth]
    sel = np.argpartition(-V, KNN_K - 1, axis=1)[:, :KNN_K]
    rowix = np.arange(B)[:, None]
    sel_v = V[rowix, sel]

    # Boundary ties -> per-query fallback (argpartition splits arbitrarily)
    vmin = sel_v.min(axis=1)
    tie = (V == vmin[:, None]).sum(axis=1) != (sel_v == vmin[:, None]).sum(axis=1)
    for q in np.nonzero(tie)[0]:
        full_fallback.add(int(q))
    LAST_INFO["fallback_rows"] = len(full_fallback)

    cand_class = np.repeat(chunk_class, 8)  # [M] class per candidate slot
    labels = cand_class[sel]  # [B, K]

    # Pathological guard: if the top-200 threshold ever sits near/below 0,
    # zero-pad dropping could hide real candidates -> recompute those rows.
    for q in np.nonzero(t1 < 1.0)[0]:
        full_fallback.add(int(q))

    scores = np.zeros((B, NUM_CLASSES), dtype=np.float32)
    with np.errstate(over="ignore"):
        w = np.exp(sel_v.astype(np.float32) / np.float32(KNN_T))
    ok = np.ones(B, dtype=bool)
    for q in full_fallback:
        ok[q] = False
    qs = np.nonzero(ok)[0]
    np.add.at(
        scores,
        (np.repeat(qs, KNN_K), labels[qs].ravel()),
        w[qs].ravel(),
    )

    for q in full_fallback:
        sims = xt64 @ x64[q]
        order = np.lexsort((np.arange(N), -sims))[:KNN_K]
        lab = y_train[order]
        with np.errstate(over="ignore"):
            wq = np.exp(sims[order].astype(np.float32) / np.float32(KNN_T))
        np.add.at(scores[q], lab, wq)

    return np.argsort(-scores, axis=1, kind="stable").astype(np.int32)


def kernel(x, x_train, y_train):
    x = np.asarray(x, dtype=np.float32)
    x_train = np.asarray(x_train, dtype=np.float32)
    y_train = np.asarray(y_train).astype(np.int64)

    colmap, chunk_class, C, groups = _plan_layout(y_train)
    nc = _get_program(C, groups)

    ncols_tot = colmap.shape[0]
    xtrP = np.zeros((D, ncols_tot), dtype=np.float32)  # padded, transposed
    real = colmap >= 0
    xtrP[:, real] = x_train.T[:, colmap[real]]

    xT = np.ascontiguousarray(x.T)
    ncols = C * CHUNK
    in_maps = [
        {
            "xT": xT,
            "wT": np.ascontiguousarray(xtrP[:, c * ncols : (c + 1) * ncols]),
        }
        for c in range(NCORES)
    ]

    res = run_bass_kernel_spmd(nc, in_maps, core_ids=list(range(NCORES)))
    LAST_INFO["exec_time_ns"] = res.exec_time_ns
    LAST_INFO["results"] = res

    vals = np.stack([res.results[c]["vals"] for c in range(NCORES)])  # [8, B, C*8]
    return _host_merge(x, x_train, y_train, vals, colmap, chunk_class, C)
